# revision 4
# baseline (speedup 1.0000x reference)
"""Trainium2 Bass kernel: fused multi-head self-attention block (CrossAttention module).

Sharding: 8 cores, each handles one (batch, query-slice) pair:
  core c -> batch b = c // 4, query rows q0 = (c % 4) * 1024 .. +1024.
Each core computes K/V projections for its full batch (replicated across the 4
cores sharing a batch), Q projection for its query slice, all 8 heads of
attention for its queries, and the output projection for its rows.
Host folds the per-channel gammas into the (pre-transposed) weights, transposes
x once, and concatenates the per-core outputs.

On-chip dataflow (per core, all fp32):
  - kT[ko, n]  = WkT.T @ xT   (key channels on partitions)  -- JIT per head-pair
  - qT[qo, n]  = WqT.T @ xTq                                -- JIT per head-pair
  - v[k, vo]   = xT.T @ WvT, stored interleaved with a ones column per head
                 ("vone" [128, 8*65]) so the attention rowsum comes free
  - scoresT tile [key 128, q 512] = kT_h.T @ qT_h, two heads packed as PE
    row-tiles (K=64 each) into one 2-bank PSUM tile
  - E = exp(SCALE * scoresT) via ScalarE, PSUM -> SBUF ([128, 1024] per instr)
  - rT[dv(+rowsum), q] += vone_h.T @ E, accumulated over 32 key chunks in PSUM
  - normalize: recip(rowsum) broadcast (GpSimd) and multiply (DVE) -> rTn
  - outT[do, q] = WoT.T @ rTn + bo
"""

import os
import sys

import numpy as np

for _p in ("/opt/trn_rl_repo", "/root/.axon_site/_ro/trn_rl_repo"):
    if os.path.isdir(_p) and _p not in sys.path:
        sys.path.append(_p)

B, N, D = 2, 4096, 512
H, DH = 8, 64
SCALE = DH ** -0.5
NCORES = 8
QPC = (B * N) // NCORES  # 1024 query rows per core
P = 128
CD = D // P              # 4 contraction chunks of 128
KC = N // P              # 32 key chunks of 128
NT = N // 512            # 8 key-column tiles of 512
QT = QPC // 512          # 2 query tiles of 512
HP = H // 2              # 4 head pairs

_PROGRAM = None
LAST_RESULT = None


def _build_program():
    import concourse.tile as tile
    from concourse import bacc, mybir

    f32 = mybir.dt.float32
    AF = mybir.ActivationFunctionType
    OP = mybir.AluOpType

    nc = bacc.Bacc("TRN2", target_bir_lowering=False, debug=False)

    xT_a = nc.dram_tensor("xT", [D, N], f32, kind="ExternalInput").ap()
    xTq_a = nc.dram_tensor("xTq", [D, QPC], f32, kind="ExternalInput").ap()
    wq_a = nc.dram_tensor("wqT", [D, D], f32, kind="ExternalInput").ap()
    wk_a = nc.dram_tensor("wkT", [D, D], f32, kind="ExternalInput").ap()
    wv_a = nc.dram_tensor("wvT", [D, D], f32, kind="ExternalInput").ap()
    wo_a = nc.dram_tensor("woT", [D, D], f32, kind="ExternalInput").ap()
    bo_a = nc.dram_tensor("bo", [D], f32, kind="ExternalInput").ap()
    outT_a = nc.dram_tensor("outT", [D, QPC], f32, kind="ExternalOutput").ap()

    with tile.TileContext(nc) as tc:
        with (
            tc.tile_pool(name="w", bufs=1) as wpool,
            tc.tile_pool(name="xs", bufs=2) as xs,
            tc.tile_pool(name="kT", bufs=2) as kTp,
            tc.tile_pool(name="qT", bufs=2) as qTp,
            tc.tile_pool(name="vone", bufs=1) as vpool,
            tc.tile_pool(name="et", bufs=2) as etp,
            tc.tile_pool(name="rTn", bufs=1) as rTnp,
            tc.tile_pool(name="ot", bufs=2) as otp,
            tc.tile_pool(name="nrm", bufs=2) as nrm,
            tc.tile_pool(name="acc", bufs=4, space="PSUM") as psa,
            tc.tile_pool(name="sc", bufs=2, space="PSUM") as pss,
        ):
            def load_w(dram_ap, tag):
                w = wpool.tile([P, CD * 512], f32, tag=tag)
                for cd in range(CD):
                    nc.sync.dma_start(
                        w[:, cd * 512:(cd + 1) * 512],
                        dram_ap[cd * P:(cd + 1) * P, :],
                    )
                return w

            wk = load_w(wk_a, "wk")
            wq = load_w(wq_a, "wq")
            wv = load_w(wv_a, "wvo")
            bo_t = wpool.tile([P, CD], f32, tag="bo")
            nc.sync.dma_start(bo_t[:], bo_a.rearrange("(c p) -> p c", p=P))

            vones = [None] * KC
            rTns = [
                rTnp.tile([P, QPC], f32, tag=f"rTn{c}", name=f"rTn{c}")
                for c in range(CD)
            ]

            def proj_group(w_t, hp, src_ap, nt, dst):
                """One 512-wide output block of a W.T @ x projection:
                4 streamed rhs tiles, 4 accumulating matmuls, 1 evacuation."""
                xts = []
                for cd in range(CD):
                    t = xs.tile([P, 512], f32, tag=f"xk{cd}")
                    nc.sync.dma_start(
                        t[:], src_ap[cd * P:(cd + 1) * P, nt * 512:(nt + 1) * 512]
                    )
                    xts.append(t)
                ps = psa.tile([P, 512], f32, tag="acc")
                for cd in range(CD):
                    nc.tensor.matmul(
                        ps[:],
                        w_t[:, cd * 512 + hp * P: cd * 512 + (hp + 1) * P],
                        xts[cd][:],
                        start=(cd == 0),
                        stop=(cd == CD - 1),
                    )
                nc.vector.tensor_copy(dst, ps[:])

            def vproj_group(kc):
                """v projection for one 128-key chunk, written into the
                per-head [64 v | 1 ones] interleaved layout."""
                xts = []
                for cd in range(CD):
                    t = xs.tile([P, P], f32, tag=f"xv{cd}")
                    nc.sync.dma_start(
                        t[:], xT_a[cd * P:(cd + 1) * P, kc * P:(kc + 1) * P]
                    )
                    xts.append(t)
                ps = psa.tile([P, 512], f32, tag="acc")
                for cd in range(CD):
                    nc.tensor.matmul(
                        ps[:],
                        xts[cd][:],
                        wv[:, cd * 512:(cd + 1) * 512],
                        start=(cd == 0),
                        stop=(cd == CD - 1),
                    )
                vt = vpool.tile([P, H * 65], f32, tag=f"vone{kc}")
                v3 = vt[:].rearrange("p (h c) -> p h c", c=65)
                nc.vector.tensor_copy(
                    v3[:, :, 0:64], ps[:].rearrange("p (h c) -> p h c", c=64)
                )
                nc.vector.memset(v3[:, :, 64:65], 1.0)
                vones[kc] = vt

            def make_proj_thunks(hp):
                qt_t = qTp.tile([P, QPC], f32, tag="qT")
                kt_t = kTp.tile([P, N], f32, tag="kT")
                thunks = []
                for nt in range(QT):
                    thunks.append(
                        lambda nt=nt, qt_t=qt_t, hp=hp: proj_group(
                            wq, hp, xTq_a, nt, qt_t[:, nt * 512:(nt + 1) * 512]
                        )
                    )
                for nt in range(NT):
                    thunks.append(
                        lambda nt=nt, kt_t=kt_t, hp=hp: proj_group(
                            wk, hp, xT_a, nt, kt_t[:, nt * 512:(nt + 1) * 512]
                        )
                    )
                return qt_t, kt_t, thunks

            qts, kts = {}, {}
            qts[0], kts[0], pending = make_proj_thunks(0)
            for t in pending:
                t()
            pending = []

            for hp in range(HP):
                qt_t, kt_t = qts[hp], kts[hp]
                for t in pending:  # leftover projections for this head pair
                    t()
                pending = []
                h0, h1 = 2 * hp, 2 * hp + 1
                for qt in range(QT):
                    rA = psa.tile([P, 512], f32, tag="acc")
                    rB = psa.tile([P, 512], f32, tag="acc")
                    if qt == 1 and hp + 1 < HP:
                        qts[hp + 1], kts[hp + 1], pending = make_proj_thunks(hp + 1)
                    qA = qt_t[0:64, qt * 512:(qt + 1) * 512]
                    qB = qt_t[64:128, qt * 512:(qt + 1) * 512]
                    for kc in range(KC):
                        if hp == 0 and qt == 0:
                            vproj_group(kc)
                        sp = pss.tile([P, 1024], f32, tag="sc")
                        nc.tensor.matmul(
                            sp[:, 0:512],
                            kt_t[0:64, kc * P:(kc + 1) * P],
                            qA,
                            start=True, stop=True,
                            tile_position=(0, 0),
                        )
                        nc.tensor.matmul(
                            sp[:, 512:1024],
                            kt_t[64:128, kc * P:(kc + 1) * P],
                            qB,
                            start=True, stop=True,
                            tile_position=(64, 0),
                        )
                        et = etp.tile([P, 1024], f32, tag="et")
                        nc.scalar.activation(et[:], sp[:], AF.Exp, scale=float(SCALE))
                        vt = vones[kc]
                        nc.tensor.matmul(
                            rA[0:65, :],
                            vt[:, h0 * 65:(h0 + 1) * 65],
                            et[:, 0:512],
                            start=(kc == 0), stop=(kc == KC - 1),
                        )
                        nc.tensor.matmul(
                            rB[0:65, :],
                            vt[:, h1 * 65:(h1 + 1) * 65],
                            et[:, 512:1024],
                            start=(kc == 0), stop=(kc == KC - 1),
                        )
                        if pending and kc % 3 == 2:
                            pending.pop(0)()
                    for r_ps, poff in ((rA, 0), (rB, 64)):
                        rc = nrm.tile([1, 512], f32, tag="rc")
                        nc.vector.reciprocal(rc[:], r_ps[64:65, :])
                        bc = nrm.tile([64, 512], f32, tag="bc")
                        nc.gpsimd.partition_broadcast(bc[:], rc[:])
                        nc.vector.tensor_tensor(
                            rTns[hp][poff:poff + 64, qt * 512:(qt + 1) * 512],
                            r_ps[0:64, :],
                            bc[:],
                            op=OP.mult,
                        )

            wo = load_w(wo_a, "wvo")  # reuses the wv slot after its last read
            for qt2 in range(QT):
                for doc in range(CD):
                    ps = psa.tile([P, 512], f32, tag="acc")
                    for cd in range(CD):
                        nc.tensor.matmul(
                            ps[:],
                            wo[:, cd * 512 + doc * P: cd * 512 + (doc + 1) * P],
                            rTns[cd][:, qt2 * 512:(qt2 + 1) * 512],
                            start=(cd == 0),
                            stop=(cd == CD - 1),
                        )
                    ot = otp.tile([P, 512], f32, tag="ot")
                    nc.vector.tensor_tensor(
                        ot[:],
                        ps[:],
                        bo_t[:, doc:doc + 1].to_broadcast((P, 512)),
                        op=OP.add,
                    )
                    nc.sync.dma_start(
                        outT_a[doc * P:(doc + 1) * P, qt2 * 512:(qt2 + 1) * 512],
                        ot[:],
                    )

    nc.compile()
    return nc


def _get_program():
    global _PROGRAM
    if _PROGRAM is None:
        _PROGRAM = _build_program()
    return _PROGRAM


def kernel(x, Wq, Wk, Wv, Wo, bo, gamma_q, gamma_k, gamma_v, gamma_out):
    from concourse import bass_utils

    x = np.asarray(x, dtype=np.float32)
    f32 = np.float32
    WqT = np.ascontiguousarray(np.asarray(Wq, f32).T * np.asarray(gamma_q, f32)[None, :])
    WkT = np.ascontiguousarray(np.asarray(Wk, f32).T * np.asarray(gamma_k, f32)[None, :])
    WvT = np.ascontiguousarray(np.asarray(Wv, f32).T * np.asarray(gamma_v, f32)[None, :])
    WoT = np.ascontiguousarray(np.asarray(Wo, f32).T * np.asarray(gamma_out, f32)[None, :])
    bo_s = np.ascontiguousarray(np.asarray(gamma_out, f32) * np.asarray(bo, f32))

    xT = np.ascontiguousarray(x.transpose(0, 2, 1))  # [B, D, N]

    in_maps = []
    for c in range(NCORES):
        b, q0 = c // 4, (c % 4) * QPC
        in_maps.append({
            "xT": xT[b],
            "xTq": np.ascontiguousarray(xT[b][:, q0:q0 + QPC]),
            "wqT": WqT, "wkT": WkT, "wvT": WvT, "woT": WoT,
            "bo": bo_s,
        })

    nc = _get_program()
    res = bass_utils.run_bass_kernel_spmd(nc, in_maps, core_ids=list(range(NCORES)))
    global LAST_RESULT
    LAST_RESULT = res

    out = np.empty((B, N, D), np.float32)
    for c in range(NCORES):
        b, q0 = c // 4, (c % 4) * QPC
        out[b, q0:q0 + QPC, :] = res.results[c]["outT"].T
    return out


# revision 8
# speedup vs baseline: 2.4370x; 2.4370x over previous
"""Trainium2 Bass kernel: fused multi-head self-attention block (CrossAttention module).

Sharding: 8 cores, each handles one (batch, query-slice) pair:
  core c -> batch b = c // 4, query rows q0 = (c % 4) * 1024 .. +1024.
Each core computes K/V projections for its full batch (replicated across the 4
cores sharing a batch), Q projection for its query slice, all 8 heads of
attention for its queries, and the output projection for its rows.
Host folds the per-channel gammas into the (pre-transposed) weights, transposes
x once, and concatenates the per-core outputs.

On-chip dataflow (per core, all fp32):
  - kT[ko, n]  = WkT.T @ xT   (key channels on partitions)  -- JIT per head-pair
  - qT[qo, n]  = WqT.T @ xTq                                -- JIT per head-pair
  - v[k, vo]   = xT.T @ WvT, stored interleaved with a ones column per head
                 ("vone" [128, 8*65]) so the attention rowsum comes free
  - scoresT tile [key 128, q 512] = kT_h.T @ qT_h, two heads packed as PE
    row-tiles (K=64 each) into one 2-bank PSUM tile
  - E = exp(SCALE * scoresT) via ScalarE, PSUM -> SBUF ([128, 1024] per instr)
  - rT[dv(+rowsum), q] += vone_h.T @ E, accumulated over 32 key chunks in PSUM
  - normalize: recip(rowsum) broadcast (GpSimd) and multiply (DVE) -> rTn
  - outT[do, q] = WoT.T @ rTn + bo
"""

import os
import sys

import numpy as np

for _p in ("/opt/trn_rl_repo", "/root/.axon_site/_ro/trn_rl_repo"):
    if os.path.isdir(_p) and _p not in sys.path:
        sys.path.append(_p)

B, N, D = 2, 4096, 512
H, DH = 8, 64
SCALE = DH ** -0.5
NCORES = 8
QPC = (B * N) // NCORES  # 1024 query rows per core
P = 128
CD = D // P              # 4 contraction chunks of 128
KC = N // P              # 32 key chunks of 128
NT = N // 512            # 8 key-column tiles of 512
QT = QPC // 512          # 2 query tiles of 512
HP = H // 2              # 4 head pairs

_PROGRAM = None
LAST_RESULT = None


def _build_program():
    import concourse.tile as tile
    from concourse import bacc, mybir

    f32 = mybir.dt.float32
    bf16 = mybir.dt.bfloat16
    AF = mybir.ActivationFunctionType
    OP = mybir.AluOpType

    nc = bacc.Bacc("TRN2", target_bir_lowering=False, debug=False)

    xT_a = nc.dram_tensor("xT", [D, N], bf16, kind="ExternalInput").ap()
    xTq_a = nc.dram_tensor("xTq", [D, QPC], bf16, kind="ExternalInput").ap()
    wq_a = nc.dram_tensor("wqT", [D, D], bf16, kind="ExternalInput").ap()
    wk_a = nc.dram_tensor("wkT", [D, D], bf16, kind="ExternalInput").ap()
    wv_a = nc.dram_tensor("wvT", [D, D], bf16, kind="ExternalInput").ap()
    wo_a = nc.dram_tensor("woT", [D, D], bf16, kind="ExternalInput").ap()
    bo_a = nc.dram_tensor("bo", [D], f32, kind="ExternalInput").ap()
    outT_a = nc.dram_tensor("outT", [D, QPC], f32, kind="ExternalOutput").ap()

    with tile.TileContext(nc) as tc:
        with (
            tc.tile_pool(name="w", bufs=1) as wpool,
            tc.tile_pool(name="xs", bufs=2) as xs,
            tc.tile_pool(name="kT", bufs=2) as kTp,
            tc.tile_pool(name="qT", bufs=2) as qTp,
            tc.tile_pool(name="vone", bufs=1) as vpool,
            tc.tile_pool(name="et", bufs=2) as etp,
            tc.tile_pool(name="rTn", bufs=1) as rTnp,
            tc.tile_pool(name="ot", bufs=2) as otp,
            tc.tile_pool(name="nrm", bufs=2) as nrm,
            tc.tile_pool(name="acc", bufs=4, space="PSUM") as psa,
            tc.tile_pool(name="sc", bufs=2, space="PSUM") as pss,
        ):
            def load_w(dram_ap, tag):
                w = wpool.tile([P, CD * 512], bf16, tag=tag)
                for cd in range(CD):
                    nc.sync.dma_start(
                        w[:, cd * 512:(cd + 1) * 512],
                        dram_ap[cd * P:(cd + 1) * P, :],
                    )
                return w

            wk = load_w(wk_a, "wk")
            wq = load_w(wq_a, "wq")
            wv = load_w(wv_a, "wvo")
            bo_t = wpool.tile([P, CD], f32, tag="bo")
            nc.sync.dma_start(bo_t[:], bo_a.rearrange("(c p) -> p c", p=P))

            vones = [None] * KC
            rTns = [
                rTnp.tile([P, QPC], bf16, tag=f"rTn{c}", name=f"rTn{c}")
                for c in range(CD)
            ]

            def proj_group(w_t, hp, src_ap, nt, dst):
                """One 512-wide output block of a W.T @ x projection:
                4 streamed rhs tiles, 4 accumulating matmuls, 1 evacuation."""
                xts = []
                for cd in range(CD):
                    t = xs.tile([P, 512], bf16, tag=f"xk{cd}")
                    nc.sync.dma_start(
                        t[:], src_ap[cd * P:(cd + 1) * P, nt * 512:(nt + 1) * 512]
                    )
                    xts.append(t)
                ps = psa.tile([P, 512], f32, tag="acc")
                for cd in range(CD):
                    nc.tensor.matmul(
                        ps[:],
                        w_t[:, cd * 512 + hp * P: cd * 512 + (hp + 1) * P],
                        xts[cd][:],
                        start=(cd == 0),
                        stop=(cd == CD - 1),
                    )
                nc.vector.tensor_copy(dst, ps[:])

            def vproj_group(kc):
                """v projection for one 128-key chunk, written into the
                per-head [64 v | 1 ones] interleaved layout."""
                xts = []
                for cd in range(CD):
                    t = xs.tile([P, P], bf16, tag=f"xv{cd}")
                    nc.sync.dma_start(
                        t[:], xT_a[cd * P:(cd + 1) * P, kc * P:(kc + 1) * P]
                    )
                    xts.append(t)
                ps = psa.tile([P, 512], f32, tag="acc")
                for cd in range(CD):
                    nc.tensor.matmul(
                        ps[:],
                        xts[cd][:],
                        wv[:, cd * 512:(cd + 1) * 512],
                        start=(cd == 0),
                        stop=(cd == CD - 1),
                    )
                vt = vpool.tile([P, H * 65], bf16, tag=f"vone{kc}")
                v3 = vt[:].rearrange("p (h c) -> p h c", c=65)
                nc.vector.tensor_copy(
                    v3[:, :, 0:64], ps[:].rearrange("p (h c) -> p h c", c=64)
                )
                nc.vector.memset(v3[:, :, 64:65], 1.0)
                vones[kc] = vt

            def make_proj_thunks(hp):
                qt_t = qTp.tile([P, QPC], bf16, tag="qT")
                kt_t = kTp.tile([P, N], bf16, tag="kT")
                thunks = []
                for nt in range(QT):
                    thunks.append(
                        lambda nt=nt, qt_t=qt_t, hp=hp: proj_group(
                            wq, hp, xTq_a, nt, qt_t[:, nt * 512:(nt + 1) * 512]
                        )
                    )
                for nt in range(NT):
                    thunks.append(
                        lambda nt=nt, kt_t=kt_t, hp=hp: proj_group(
                            wk, hp, xT_a, nt, kt_t[:, nt * 512:(nt + 1) * 512]
                        )
                    )
                return qt_t, kt_t, thunks

            qts, kts = {}, {}
            qts[0], kts[0], pending = make_proj_thunks(0)
            for t in pending:
                t()
            pending = []

            for hp in range(HP):
                qt_t, kt_t = qts[hp], kts[hp]
                for t in pending:  # leftover projections for this head pair
                    t()
                pending = []
                h0, h1 = 2 * hp, 2 * hp + 1
                for qt in range(QT):
                    rA = psa.tile([P, 512], f32, tag="acc")
                    rB = psa.tile([P, 512], f32, tag="acc")
                    if qt == 1 and hp + 1 < HP:
                        qts[hp + 1], kts[hp + 1], pending = make_proj_thunks(hp + 1)
                    qA = qt_t[0:64, qt * 512:(qt + 1) * 512]
                    qB = qt_t[64:128, qt * 512:(qt + 1) * 512]
                    for kc in range(KC):
                        if hp == 0 and qt == 0:
                            vproj_group(kc)
                        sp = pss.tile([P, 1024], f32, tag="sc")
                        nc.tensor.matmul(
                            sp[:, 0:512],
                            kt_t[0:64, kc * P:(kc + 1) * P],
                            qA,
                            start=True, stop=True,
                            tile_position=(0, 0),
                        )
                        nc.tensor.matmul(
                            sp[:, 512:1024],
                            kt_t[64:128, kc * P:(kc + 1) * P],
                            qB,
                            start=True, stop=True,
                            tile_position=(64, 0),
                        )
                        et = etp.tile([P, 1024], bf16, tag="et")
                        nc.scalar.activation(et[:], sp[:], AF.Exp, scale=float(SCALE))
                        vt = vones[kc]
                        nc.tensor.matmul(
                            rA[0:65, :],
                            vt[:, h0 * 65:(h0 + 1) * 65],
                            et[:, 0:512],
                            start=(kc == 0), stop=(kc == KC - 1),
                        )
                        nc.tensor.matmul(
                            rB[0:65, :],
                            vt[:, h1 * 65:(h1 + 1) * 65],
                            et[:, 512:1024],
                            start=(kc == 0), stop=(kc == KC - 1),
                        )
                        if pending and kc % 3 == 2:
                            pending.pop(0)()
                    for r_ps, poff in ((rA, 0), (rB, 64)):
                        rc = nrm.tile([1, 512], f32, tag="rc")
                        nc.vector.reciprocal(rc[:], r_ps[64:65, :])
                        bc = nrm.tile([64, 512], f32, tag="bc")
                        nc.gpsimd.partition_broadcast(bc[:], rc[:])
                        nc.vector.tensor_tensor(
                            rTns[hp][poff:poff + 64, qt * 512:(qt + 1) * 512],
                            r_ps[0:64, :],
                            bc[:],
                            op=OP.mult,
                        )

            wo = load_w(wo_a, "wvo")  # reuses the wv slot after its last read
            for qt2 in range(QT):
                for doc in range(CD):
                    ps = psa.tile([P, 512], f32, tag="acc")
                    for cd in range(CD):
                        nc.tensor.matmul(
                            ps[:],
                            wo[:, cd * 512 + doc * P: cd * 512 + (doc + 1) * P],
                            rTns[cd][:, qt2 * 512:(qt2 + 1) * 512],
                            start=(cd == 0),
                            stop=(cd == CD - 1),
                        )
                    ot = otp.tile([P, 512], f32, tag="ot")
                    nc.vector.tensor_tensor(
                        ot[:],
                        ps[:],
                        bo_t[:, doc:doc + 1].to_broadcast((P, 512)),
                        op=OP.add,
                    )
                    nc.sync.dma_start(
                        outT_a[doc * P:(doc + 1) * P, qt2 * 512:(qt2 + 1) * 512],
                        ot[:],
                    )

    nc.compile()
    return nc


def _get_program():
    global _PROGRAM
    if _PROGRAM is None:
        _PROGRAM = _build_program()
    return _PROGRAM


def kernel(x, Wq, Wk, Wv, Wo, bo, gamma_q, gamma_k, gamma_v, gamma_out):
    from concourse import bass_utils

    import ml_dtypes

    bf16 = ml_dtypes.bfloat16
    x = np.asarray(x, dtype=np.float32)
    f32 = np.float32
    WqT = np.ascontiguousarray((np.asarray(Wq, f32).T * np.asarray(gamma_q, f32)[None, :]).astype(bf16))
    WkT = np.ascontiguousarray((np.asarray(Wk, f32).T * np.asarray(gamma_k, f32)[None, :]).astype(bf16))
    WvT = np.ascontiguousarray((np.asarray(Wv, f32).T * np.asarray(gamma_v, f32)[None, :]).astype(bf16))
    WoT = np.ascontiguousarray((np.asarray(Wo, f32).T * np.asarray(gamma_out, f32)[None, :]).astype(bf16))
    bo_s = np.ascontiguousarray(np.asarray(gamma_out, f32) * np.asarray(bo, f32))

    xT = np.ascontiguousarray(x.transpose(0, 2, 1).astype(bf16))  # [B, D, N]

    in_maps = []
    for c in range(NCORES):
        b, q0 = c // 4, (c % 4) * QPC
        in_maps.append({
            "xT": xT[b],
            "xTq": np.ascontiguousarray(xT[b][:, q0:q0 + QPC]),
            "wqT": WqT, "wkT": WkT, "wvT": WvT, "woT": WoT,
            "bo": bo_s,
        })

    nc = _get_program()
    res = bass_utils.run_bass_kernel_spmd(nc, in_maps, core_ids=list(range(NCORES)))
    global LAST_RESULT
    LAST_RESULT = res

    out = np.empty((B, N, D), np.float32)
    for c in range(NCORES):
        b, q0 = c // 4, (c % 4) * QPC
        out[b, q0:q0 + QPC, :] = res.results[c]["outT"].T
    return out


# revision 10
# speedup vs baseline: 2.4704x; 1.0137x over previous
"""Trainium2 Bass kernel: fused multi-head self-attention block (CrossAttention module).

Sharding: 8 cores, each handles one (batch, query-slice) pair:
  core c -> batch b = c // 4, query rows q0 = (c % 4) * 1024 .. +1024.
Each core computes K/V projections for its full batch (replicated across the 4
cores sharing a batch), Q projection for its query slice, all 8 heads of
attention for its queries, and the output projection for its rows.
Host folds the per-channel gammas into the (pre-transposed) weights, transposes
x once, and concatenates the per-core outputs.

On-chip dataflow (per core, all fp32):
  - kT[ko, n]  = WkT.T @ xT   (key channels on partitions)  -- JIT per head-pair
  - qT[qo, n]  = WqT.T @ xTq                                -- JIT per head-pair
  - v[k, vo]   = xT.T @ WvT, stored interleaved with a ones column per head
                 ("vone" [128, 8*65]) so the attention rowsum comes free
  - scoresT tile [key 128, q 512] = kT_h.T @ qT_h, two heads packed as PE
    row-tiles (K=64 each) into one 2-bank PSUM tile
  - E = exp(SCALE * scoresT) via ScalarE, PSUM -> SBUF ([128, 1024] per instr)
  - rT[dv(+rowsum), q] += vone_h.T @ E, accumulated over 32 key chunks in PSUM
  - normalize: recip(rowsum) broadcast (GpSimd) and multiply (DVE) -> rTn
  - outT[do, q] = WoT.T @ rTn + bo
"""

import os
import sys

import numpy as np

for _p in ("/opt/trn_rl_repo", "/root/.axon_site/_ro/trn_rl_repo"):
    if os.path.isdir(_p) and _p not in sys.path:
        sys.path.append(_p)

B, N, D = 2, 4096, 512
H, DH = 8, 64
SCALE = DH ** -0.5
NCORES = 8
QPC = (B * N) // NCORES  # 1024 query rows per core
P = 128
CD = D // P              # 4 contraction chunks of 128
KC = N // P              # 32 key chunks of 128
NT = N // 512            # 8 key-column tiles of 512
QT = QPC // 512          # 2 query tiles of 512
HP = H // 2              # 4 head pairs

_PROGRAM = None
LAST_RESULT = None


def _build_program():
    import concourse.tile as tile
    from concourse import bacc, mybir

    f32 = mybir.dt.float32
    bf16 = mybir.dt.bfloat16
    AF = mybir.ActivationFunctionType
    OP = mybir.AluOpType

    nc = bacc.Bacc("TRN2", target_bir_lowering=False, debug=False)

    xT_a = nc.dram_tensor("xT", [D, N], bf16, kind="ExternalInput").ap()
    xTq_a = nc.dram_tensor("xTq", [D, QPC], bf16, kind="ExternalInput").ap()
    wq_a = nc.dram_tensor("wqT", [D, D], bf16, kind="ExternalInput").ap()
    wk_a = nc.dram_tensor("wkT", [D, D], bf16, kind="ExternalInput").ap()
    wv_a = nc.dram_tensor("wvT", [D, D], bf16, kind="ExternalInput").ap()
    wo_a = nc.dram_tensor("woT", [D, D], bf16, kind="ExternalInput").ap()
    bo_a = nc.dram_tensor("bo", [D], f32, kind="ExternalInput").ap()
    outT_a = nc.dram_tensor("outT", [D, QPC], f32, kind="ExternalOutput").ap()

    with tile.TileContext(nc) as tc:
        with (
            tc.tile_pool(name="w", bufs=1) as wpool,
            tc.tile_pool(name="xs", bufs=2) as xs,
            tc.tile_pool(name="kT", bufs=2) as kTp,
            tc.tile_pool(name="qT", bufs=2) as qTp,
            tc.tile_pool(name="vone", bufs=1) as vpool,
            tc.tile_pool(name="et", bufs=4) as etp,
            tc.tile_pool(name="rTn", bufs=1) as rTnp,
            tc.tile_pool(name="ot", bufs=2) as otp,
            tc.tile_pool(name="nrm", bufs=2) as nrm,
            tc.tile_pool(name="acc", bufs=4, space="PSUM") as psa,
            tc.tile_pool(name="sc", bufs=2, space="PSUM") as pss,
        ):
            def load_w(dram_ap, tag):
                w = wpool.tile([P, CD * 512], bf16, tag=tag)
                for cd in range(CD):
                    nc.sync.dma_start(
                        w[:, cd * 512:(cd + 1) * 512],
                        dram_ap[cd * P:(cd + 1) * P, :],
                    )
                return w

            wk = load_w(wk_a, "wk")
            wq = load_w(wq_a, "wq")
            wv = load_w(wv_a, "wvo")
            bo_t = wpool.tile([P, CD], f32, tag="bo")
            nc.sync.dma_start(bo_t[:], bo_a.rearrange("(c p) -> p c", p=P))

            vones = [None] * KC
            rTns = [
                rTnp.tile([P, QPC], bf16, tag=f"rTn{c}", name=f"rTn{c}")
                for c in range(CD)
            ]

            def proj_group(w_t, hp, src_ap, nt, dst):
                """One 512-wide output block of a W.T @ x projection:
                4 streamed rhs tiles, 4 accumulating matmuls, 1 evacuation."""
                xts = []
                for cd in range(CD):
                    t = xs.tile([P, 512], bf16, tag=f"xk{cd}")
                    nc.sync.dma_start(
                        t[:], src_ap[cd * P:(cd + 1) * P, nt * 512:(nt + 1) * 512]
                    )
                    xts.append(t)
                ps = psa.tile([P, 512], f32, tag="acc")
                for cd in range(CD):
                    nc.tensor.matmul(
                        ps[:],
                        w_t[:, cd * 512 + hp * P: cd * 512 + (hp + 1) * P],
                        xts[cd][:],
                        start=(cd == 0),
                        stop=(cd == CD - 1),
                    )
                nc.vector.tensor_copy(dst, ps[:])

            def vproj_group(kc):
                """v projection for one 128-key chunk, written into the
                per-head [64 v | 1 ones] interleaved layout."""
                xts = []
                for cd in range(CD):
                    t = xs.tile([P, P], bf16, tag=f"xv{cd}")
                    nc.sync.dma_start(
                        t[:], xT_a[cd * P:(cd + 1) * P, kc * P:(kc + 1) * P]
                    )
                    xts.append(t)
                ps = psa.tile([P, 512], f32, tag="acc")
                for cd in range(CD):
                    nc.tensor.matmul(
                        ps[:],
                        xts[cd][:],
                        wv[:, cd * 512:(cd + 1) * 512],
                        start=(cd == 0),
                        stop=(cd == CD - 1),
                    )
                vt = vpool.tile([P, H * 65], bf16, tag=f"vone{kc}")
                v3 = vt[:].rearrange("p (h c) -> p h c", c=65)
                nc.vector.tensor_copy(
                    v3[:, :, 0:64], ps[:].rearrange("p (h c) -> p h c", c=64)
                )
                nc.vector.memset(v3[:, :, 64:65], 1.0)
                vones[kc] = vt

            def make_proj_thunks(hp):
                qt_t = qTp.tile([P, QPC], bf16, tag="qT")
                kt_t = kTp.tile([P, N], bf16, tag="kT")
                thunks = []
                for nt in range(QT):
                    thunks.append(
                        lambda nt=nt, qt_t=qt_t, hp=hp: proj_group(
                            wq, hp, xTq_a, nt, qt_t[:, nt * 512:(nt + 1) * 512]
                        )
                    )
                for nt in range(NT):
                    thunks.append(
                        lambda nt=nt, kt_t=kt_t, hp=hp: proj_group(
                            wk, hp, xT_a, nt, kt_t[:, nt * 512:(nt + 1) * 512]
                        )
                    )
                return qt_t, kt_t, thunks

            qts, kts = {}, {}
            qts[0], kts[0], pending = make_proj_thunks(0)
            for t in pending:
                t()
            pending = []

            for hp in range(HP):
                qt_t, kt_t = qts[hp], kts[hp]
                for t in pending:  # leftover projections for this head pair
                    t()
                pending = []
                h0, h1 = 2 * hp, 2 * hp + 1
                for qt in range(QT):
                    rA = psa.tile([P, 512], f32, tag="acc")
                    rB = psa.tile([P, 512], f32, tag="acc")
                    if qt == 1 and hp + 1 < HP:
                        qts[hp + 1], kts[hp + 1], pending = make_proj_thunks(hp + 1)
                    qA = qt_t[0:64, qt * 512:(qt + 1) * 512]
                    qB = qt_t[64:128, qt * 512:(qt + 1) * 512]
                    # Software-pipelined by one chunk: emit scores(kc) and its
                    # exp, then the AV matmuls for kc-1 — so the PE always has
                    # independent score work queued while ScalarE runs exp.
                    ets = {}

                    def av_pair(kc):
                        vt = vones[kc]
                        et = ets.pop(kc)
                        nc.tensor.matmul(
                            rA[0:65, :],
                            vt[:, h0 * 65:(h0 + 1) * 65],
                            et[:, 0:512],
                            start=(kc == 0), stop=(kc == KC - 1),
                        )
                        nc.tensor.matmul(
                            rB[0:65, :],
                            vt[:, h1 * 65:(h1 + 1) * 65],
                            et[:, 512:1024],
                            start=(kc == 0), stop=(kc == KC - 1),
                        )

                    for kc in range(KC):
                        if hp == 0 and qt == 0:
                            vproj_group(kc)
                        sp = pss.tile([P, 1024], f32, tag="sc")
                        nc.tensor.matmul(
                            sp[:, 0:512],
                            kt_t[0:64, kc * P:(kc + 1) * P],
                            qA,
                            start=True, stop=True,
                            tile_position=(0, 0),
                        )
                        nc.tensor.matmul(
                            sp[:, 512:1024],
                            kt_t[64:128, kc * P:(kc + 1) * P],
                            qB,
                            start=True, stop=True,
                            tile_position=(64, 0),
                        )
                        et = etp.tile([P, 1024], bf16, tag="et")
                        nc.scalar.activation(et[:], sp[:], AF.Exp, scale=float(SCALE))
                        ets[kc] = et
                        if kc >= 1:
                            av_pair(kc - 1)
                        if pending and kc % 3 == 2:
                            pending.pop(0)()
                    av_pair(KC - 1)
                    for r_ps, poff in ((rA, 0), (rB, 64)):
                        rc = nrm.tile([1, 512], f32, tag="rc")
                        nc.vector.reciprocal(rc[:], r_ps[64:65, :])
                        bc = nrm.tile([64, 512], f32, tag="bc")
                        nc.gpsimd.partition_broadcast(bc[:], rc[:])
                        nc.vector.tensor_tensor(
                            rTns[hp][poff:poff + 64, qt * 512:(qt + 1) * 512],
                            r_ps[0:64, :],
                            bc[:],
                            op=OP.mult,
                        )

            wo = load_w(wo_a, "wvo")  # reuses the wv slot after its last read
            for qt2 in range(QT):
                for doc in range(CD):
                    ps = psa.tile([P, 512], f32, tag="acc")
                    for cd in range(CD):
                        nc.tensor.matmul(
                            ps[:],
                            wo[:, cd * 512 + doc * P: cd * 512 + (doc + 1) * P],
                            rTns[cd][:, qt2 * 512:(qt2 + 1) * 512],
                            start=(cd == 0),
                            stop=(cd == CD - 1),
                        )
                    ot = otp.tile([P, 512], f32, tag="ot")
                    nc.vector.tensor_tensor(
                        ot[:],
                        ps[:],
                        bo_t[:, doc:doc + 1].to_broadcast((P, 512)),
                        op=OP.add,
                    )
                    nc.sync.dma_start(
                        outT_a[doc * P:(doc + 1) * P, qt2 * 512:(qt2 + 1) * 512],
                        ot[:],
                    )

    nc.compile()
    return nc


def _get_program():
    global _PROGRAM
    if _PROGRAM is None:
        _PROGRAM = _build_program()
    return _PROGRAM


def kernel(x, Wq, Wk, Wv, Wo, bo, gamma_q, gamma_k, gamma_v, gamma_out):
    from concourse import bass_utils

    import ml_dtypes

    bf16 = ml_dtypes.bfloat16
    x = np.asarray(x, dtype=np.float32)
    f32 = np.float32
    WqT = np.ascontiguousarray((np.asarray(Wq, f32).T * np.asarray(gamma_q, f32)[None, :]).astype(bf16))
    WkT = np.ascontiguousarray((np.asarray(Wk, f32).T * np.asarray(gamma_k, f32)[None, :]).astype(bf16))
    WvT = np.ascontiguousarray((np.asarray(Wv, f32).T * np.asarray(gamma_v, f32)[None, :]).astype(bf16))
    WoT = np.ascontiguousarray((np.asarray(Wo, f32).T * np.asarray(gamma_out, f32)[None, :]).astype(bf16))
    bo_s = np.ascontiguousarray(np.asarray(gamma_out, f32) * np.asarray(bo, f32))

    xT = np.ascontiguousarray(x.transpose(0, 2, 1).astype(bf16))  # [B, D, N]

    in_maps = []
    for c in range(NCORES):
        b, q0 = c // 4, (c % 4) * QPC
        in_maps.append({
            "xT": xT[b],
            "xTq": np.ascontiguousarray(xT[b][:, q0:q0 + QPC]),
            "wqT": WqT, "wkT": WkT, "wvT": WvT, "woT": WoT,
            "bo": bo_s,
        })

    nc = _get_program()
    res = bass_utils.run_bass_kernel_spmd(nc, in_maps, core_ids=list(range(NCORES)))
    global LAST_RESULT
    LAST_RESULT = res

    out = np.empty((B, N, D), np.float32)
    for c in range(NCORES):
        b, q0 = c // 4, (c % 4) * QPC
        out[b, q0:q0 + QPC, :] = res.results[c]["outT"].T
    return out


# revision 13
# speedup vs baseline: 2.5481x; 1.0315x over previous
"""Trainium2 Bass kernel: fused multi-head self-attention block (CrossAttention module).

Sharding: 8 cores, each handles one (batch, query-slice) pair:
  core c -> batch b = c // 4, query rows q0 = (c % 4) * 1024 .. +1024.
Each core computes K/V projections for its full batch (replicated across the 4
cores sharing a batch), Q projection for its query slice, all 8 heads of
attention for its queries, and the output projection for its rows.
Host folds the per-channel gammas into the (pre-transposed) weights, transposes
x once, and concatenates the per-core outputs.

On-chip dataflow (per core, all fp32):
  - kT[ko, n]  = WkT.T @ xT   (key channels on partitions)  -- JIT per head-pair
  - qT[qo, n]  = WqT.T @ xTq                                -- JIT per head-pair
  - v[k, vo]   = xT.T @ WvT, stored interleaved with a ones column per head
                 ("vone" [128, 8*65]) so the attention rowsum comes free
  - scoresT tile [key 128, q 512] = kT_h.T @ qT_h, two heads packed as PE
    row-tiles (K=64 each) into one 2-bank PSUM tile
  - E = exp(SCALE * scoresT) via ScalarE, PSUM -> SBUF ([128, 1024] per instr)
  - rT[dv(+rowsum), q] += vone_h.T @ E, accumulated over 32 key chunks in PSUM
  - normalize: recip(rowsum) broadcast (GpSimd) and multiply (DVE) -> rTn
  - outT[do, q] = WoT.T @ rTn + bo
"""

import os
import sys

import numpy as np

for _p in ("/opt/trn_rl_repo", "/root/.axon_site/_ro/trn_rl_repo"):
    if os.path.isdir(_p) and _p not in sys.path:
        sys.path.append(_p)

B, N, D = 2, 4096, 512
H, DH = 8, 64
SCALE = DH ** -0.5
NCORES = 8
QPC = (B * N) // NCORES  # 1024 query rows per core
P = 128
CD = D // P              # 4 contraction chunks of 128
KC = N // P              # 32 key chunks of 128
NT = N // 512            # 8 key-column tiles of 512
QT = QPC // 512          # 2 query tiles of 512
HP = H // 2              # 4 head pairs

_PROGRAM = None
LAST_RESULT = None


def _build_program():
    import concourse.tile as tile
    from concourse import bacc, mybir

    f32 = mybir.dt.float32
    bf16 = mybir.dt.bfloat16
    AF = mybir.ActivationFunctionType
    OP = mybir.AluOpType

    nc = bacc.Bacc("TRN2", target_bir_lowering=False, debug=False)

    xT_a = nc.dram_tensor("xT", [D, N], bf16, kind="ExternalInput").ap()
    xTq_a = nc.dram_tensor("xTq", [D, QPC], bf16, kind="ExternalInput").ap()
    wq_a = nc.dram_tensor("wqT", [D, D], bf16, kind="ExternalInput").ap()
    wk_a = nc.dram_tensor("wkT", [D, D], bf16, kind="ExternalInput").ap()
    wv_a = nc.dram_tensor("wvT", [D, D], bf16, kind="ExternalInput").ap()
    wo_a = nc.dram_tensor("woT", [D, D], bf16, kind="ExternalInput").ap()
    bo_a = nc.dram_tensor("bo", [D], f32, kind="ExternalInput").ap()
    outT_a = nc.dram_tensor("outT", [D, QPC], f32, kind="ExternalOutput").ap()

    with tile.TileContext(nc) as tc:
        with (
            tc.tile_pool(name="w", bufs=1) as wpool,
            tc.tile_pool(name="xs", bufs=2) as xs,
            tc.tile_pool(name="kT", bufs=2) as kTp,
            tc.tile_pool(name="qT", bufs=2) as qTp,
            tc.tile_pool(name="vone", bufs=1) as vpool,
            tc.tile_pool(name="et", bufs=4) as etp,
            tc.tile_pool(name="rTn", bufs=1) as rTnp,
            tc.tile_pool(name="ot", bufs=2) as otp,
            tc.tile_pool(name="nrm", bufs=2) as nrm,
            tc.tile_pool(name="acc", bufs=4, space="PSUM") as psa,
            tc.tile_pool(name="sc", bufs=2, space="PSUM") as pss,
        ):
            def load_w(dram_ap, tag):
                w = wpool.tile([P, CD * 512], bf16, tag=tag)
                for cd in range(CD):
                    nc.sync.dma_start(
                        w[:, cd * 512:(cd + 1) * 512],
                        dram_ap[cd * P:(cd + 1) * P, :],
                    )
                return w

            wk = load_w(wk_a, "wk")
            wq = load_w(wq_a, "wq")
            wv = load_w(wv_a, "wvo")
            bo_t = wpool.tile([P, CD], f32, tag="bo")
            nc.sync.dma_start(bo_t[:], bo_a.rearrange("(c p) -> p c", p=P))

            vones = [None] * KC
            rTns = [
                rTnp.tile([P, QPC], bf16, tag=f"rTn{c}", name=f"rTn{c}")
                for c in range(CD)
            ]

            def proj_group(w_t, hp, src_ap, nt, dst):
                """One 512-wide output block of a W.T @ x projection:
                4 streamed rhs tiles, 4 accumulating matmuls, 1 evacuation."""
                xts = []
                for cd in range(CD):
                    t = xs.tile([P, 512], bf16, tag=f"xk{cd}")
                    nc.sync.dma_start(
                        t[:], src_ap[cd * P:(cd + 1) * P, nt * 512:(nt + 1) * 512]
                    )
                    xts.append(t)
                ps = psa.tile([P, 512], f32, tag="acc")
                for cd in range(CD):
                    nc.tensor.matmul(
                        ps[:],
                        w_t[:, cd * 512 + hp * P: cd * 512 + (hp + 1) * P],
                        xts[cd][:],
                        start=(cd == 0),
                        stop=(cd == CD - 1),
                    )
                nc.vector.tensor_copy(dst, ps[:])

            def vproj_group(kc):
                """v projection for one 128-key chunk, written into the
                per-head [64 v | 1 ones] interleaved layout."""
                xts = []
                for cd in range(CD):
                    t = xs.tile([P, P], bf16, tag=f"xv{cd}")
                    nc.sync.dma_start(
                        t[:], xT_a[cd * P:(cd + 1) * P, kc * P:(kc + 1) * P]
                    )
                    xts.append(t)
                ps = psa.tile([P, 512], f32, tag="acc")
                for cd in range(CD):
                    nc.tensor.matmul(
                        ps[:],
                        xts[cd][:],
                        wv[:, cd * 512:(cd + 1) * 512],
                        start=(cd == 0),
                        stop=(cd == CD - 1),
                    )
                vt = vpool.tile([P, H * 65], bf16, tag=f"vone{kc}")
                v3 = vt[:].rearrange("p (h c) -> p h c", c=65)
                nc.vector.tensor_copy(
                    v3[:, :, 0:64], ps[:].rearrange("p (h c) -> p h c", c=64)
                )
                nc.vector.memset(v3[:, :, 64:65], 1.0)
                vones[kc] = vt

            def make_proj_thunks(hp):
                qt_t = qTp.tile([P, QPC], bf16, tag="qT")
                kt_t = kTp.tile([P, N], bf16, tag="kT")
                thunks = []
                for nt in range(QT):
                    thunks.append(
                        lambda nt=nt, qt_t=qt_t, hp=hp: proj_group(
                            wq, hp, xTq_a, nt, qt_t[:, nt * 512:(nt + 1) * 512]
                        )
                    )
                for nt in range(NT):
                    thunks.append(
                        lambda nt=nt, kt_t=kt_t, hp=hp: proj_group(
                            wk, hp, xT_a, nt, kt_t[:, nt * 512:(nt + 1) * 512]
                        )
                    )
                return qt_t, kt_t, thunks

            qts, kts = {}, {}
            qts[0], kts[0], th0 = make_proj_thunks(0)
            # Emit only the blocks needed to start attention: qT block 0 and
            # kT block 0; the rest of hp0's projections interleave into the
            # first kc loop (kT block g must land before kc reaches 4g).
            th0[0]()
            th0[QT]()
            hp0_qt1_proj = th0[1:QT]
            hp0_kt = th0[QT + 1:]
            pending = []

            for hp in range(HP):
                qt_t, kt_t = qts[hp], kts[hp]
                for t in pending:  # leftover projections for this head pair
                    t()
                pending = []
                h0, h1 = 2 * hp, 2 * hp + 1
                for qt in range(QT):
                    rA = psa.tile([P, 512], f32, tag="acc")
                    rB = psa.tile([P, 512], f32, tag="acc")
                    if qt == 1 and hp + 1 < HP:
                        qts[hp + 1], kts[hp + 1], pending = make_proj_thunks(hp + 1)
                    qA = qt_t[0:64, qt * 512:(qt + 1) * 512]
                    qB = qt_t[64:128, qt * 512:(qt + 1) * 512]
                    # Software-pipelined by one chunk: emit scores(kc) and its
                    # exp, then the AV matmuls for kc-1 — so the PE always has
                    # independent score work queued while ScalarE runs exp.
                    ets = {}

                    def av_pair(kc):
                        vt = vones[kc]
                        et = ets.pop(kc)
                        nc.tensor.matmul(
                            rA[0:65, :],
                            vt[:, h0 * 65:(h0 + 1) * 65],
                            et[:, 0:512],
                            start=(kc == 0), stop=(kc == KC - 1),
                        )
                        nc.tensor.matmul(
                            rB[0:65, :],
                            vt[:, h1 * 65:(h1 + 1) * 65],
                            et[:, 512:1024],
                            start=(kc == 0), stop=(kc == KC - 1),
                        )

                    for kc in range(KC):
                        if hp == 0 and qt == 0:
                            vproj_group(kc)
                            if hp0_kt and kc % 4 == 2:
                                hp0_kt.pop(0)()
                            if kc == 24:
                                for t in hp0_qt1_proj:
                                    t()
                                hp0_qt1_proj = []
                        sp = pss.tile([P, 1024], f32, tag="sc")
                        nc.tensor.matmul(
                            sp[:, 0:512],
                            kt_t[0:64, kc * P:(kc + 1) * P],
                            qA,
                            start=True, stop=True,
                            tile_position=(0, 0),
                        )
                        nc.tensor.matmul(
                            sp[:, 512:1024],
                            kt_t[64:128, kc * P:(kc + 1) * P],
                            qB,
                            start=True, stop=True,
                            tile_position=(64, 0),
                        )
                        et = etp.tile([P, 1024], bf16, tag="et")
                        nc.scalar.activation(et[:], sp[:], AF.Exp, scale=float(SCALE))
                        ets[kc] = et
                        if kc >= 1:
                            av_pair(kc - 1)
                        if pending and kc % 3 == 2:
                            pending.pop(0)()
                    av_pair(KC - 1)
                    # Evacuate both accumulators to SBUF immediately (frees the
                    # PSUM slots in ~0.7us each); the slow reciprocal/broadcast/
                    # multiply normalization then runs off the critical path.
                    rsbs = []
                    for r_ps in (rA, rB):
                        rsb = nrm.tile([65, 512], f32, tag="rsb", bufs=4)
                        nc.vector.tensor_copy(rsb[:], r_ps[0:65, :])
                        rsbs.append(rsb)
                    for rsb, poff in zip(rsbs, (0, 64)):
                        rc = nrm.tile([1, 512], f32, tag="rc")
                        nc.vector.reciprocal(rc[:], rsb[64:65, :])
                        bc = nrm.tile([64, 512], f32, tag="bc")
                        nc.gpsimd.partition_broadcast(bc[:], rc[:])
                        nc.vector.tensor_tensor(
                            rTns[hp][poff:poff + 64, qt * 512:(qt + 1) * 512],
                            rsb[0:64, :],
                            bc[:],
                            op=OP.mult,
                        )

            wo = load_w(wo_a, "wvo")  # reuses the wv slot after its last read
            for qt2 in range(QT):
                for doc in range(CD):
                    ps = psa.tile([P, 512], f32, tag="acc")
                    for cd in range(CD):
                        nc.tensor.matmul(
                            ps[:],
                            wo[:, cd * 512 + doc * P: cd * 512 + (doc + 1) * P],
                            rTns[cd][:, qt2 * 512:(qt2 + 1) * 512],
                            start=(cd == 0),
                            stop=(cd == CD - 1),
                        )
                    ot = otp.tile([P, 512], f32, tag="ot")
                    nc.vector.tensor_tensor(
                        ot[:],
                        ps[:],
                        bo_t[:, doc:doc + 1].to_broadcast((P, 512)),
                        op=OP.add,
                    )
                    nc.sync.dma_start(
                        outT_a[doc * P:(doc + 1) * P, qt2 * 512:(qt2 + 1) * 512],
                        ot[:],
                    )

    nc.compile()
    return nc


def _get_program():
    global _PROGRAM
    if _PROGRAM is None:
        _PROGRAM = _build_program()
    return _PROGRAM


def kernel(x, Wq, Wk, Wv, Wo, bo, gamma_q, gamma_k, gamma_v, gamma_out):
    from concourse import bass_utils

    import ml_dtypes

    bf16 = ml_dtypes.bfloat16
    x = np.asarray(x, dtype=np.float32)
    f32 = np.float32
    WqT = np.ascontiguousarray((np.asarray(Wq, f32).T * np.asarray(gamma_q, f32)[None, :]).astype(bf16))
    WkT = np.ascontiguousarray((np.asarray(Wk, f32).T * np.asarray(gamma_k, f32)[None, :]).astype(bf16))
    WvT = np.ascontiguousarray((np.asarray(Wv, f32).T * np.asarray(gamma_v, f32)[None, :]).astype(bf16))
    WoT = np.ascontiguousarray((np.asarray(Wo, f32).T * np.asarray(gamma_out, f32)[None, :]).astype(bf16))
    bo_s = np.ascontiguousarray(np.asarray(gamma_out, f32) * np.asarray(bo, f32))

    xT = np.ascontiguousarray(x.transpose(0, 2, 1).astype(bf16))  # [B, D, N]

    in_maps = []
    for c in range(NCORES):
        b, q0 = c // 4, (c % 4) * QPC
        in_maps.append({
            "xT": xT[b],
            "xTq": np.ascontiguousarray(xT[b][:, q0:q0 + QPC]),
            "wqT": WqT, "wkT": WkT, "wvT": WvT, "woT": WoT,
            "bo": bo_s,
        })

    nc = _get_program()
    res = bass_utils.run_bass_kernel_spmd(nc, in_maps, core_ids=list(range(NCORES)))
    global LAST_RESULT
    LAST_RESULT = res

    out = np.empty((B, N, D), np.float32)
    for c in range(NCORES):
        b, q0 = c // 4, (c % 4) * QPC
        out[b, q0:q0 + QPC, :] = res.results[c]["outT"].T
    return out
